# revision 25
# baseline (speedup 1.0000x reference)
"""Multi-Head Latent Attention (MLA) forward on 8 Trainium2 NeuronCores.

Sharding: tensor-parallel over heads (16 heads -> 2 per core). Each core:
  - loads x pre-transposed (host ships x.T in bf16),
  - computes q projections for its heads plus a 1/8 column slice of the
    latent-kv encoding; per-block AllGather assembles the full latent
    (hidden behind compute),
  - RMS-norms the latent (norm weight folded into wkv_b on host, per-token
    scale applied after the up-projection),
  - applies RoPE with host-precomputed cos/sin tables,
  - runs causal attention for its 2 heads in transposed-score layout
    (scores St[k, q]; softmax without max subtraction - scores are O(1)),
  - per-batch AllToAll exchanges head outputs so each core holds all
    features for a token slice, then computes that slice of the wo
    projection from a SBUF-resident bf16 copy of wo.
Batch 1's projection work is interleaved into batch 0's attention phase and
wo(0) into batch 1's phase so the PE never idles across the collectives.
Output slices are disjoint; the host just concatenates them.
"""
import sys

if "/opt/trn_rl_repo" not in sys.path:
    sys.path.insert(0, "/opt/trn_rl_repo")

import numpy as np
import ml_dtypes
import concourse.bacc as bacc
import concourse.mybir as mybir
from concourse import tile
from concourse.bass_utils import run_bass_kernel_spmd

H, NOPE, ROPE, VD, KVR, QKD = 16, 128, 64, 128, 512, 192
B, T, D = 2, 2048, 2048
NCORES, HPC, BLK = 8, 2, 512
KVC = KVR + ROPE  # 576 latent+rope columns
KVS = KVC // NCORES  # 72-column slice per core
W1N = HPC * QKD + KVS  # 456 projection columns per core
f32 = mybir.dt.float32
f32r = mybir.dt.float32r
bf16 = mybir.dt.bfloat16
EXP = mybir.ActivationFunctionType.Exp
LN = mybir.ActivationFunctionType.Ln
SQUARE = mybir.ActivationFunctionType.Square


def r32(ap):
    return ap.bitcast(f32r)


def _patch_act_tables():
    """Make the act-table-load pass serve Exp/Ln/Square from the one set that
    contains them all (natural_log_exp_and_others), so interleaved activations
    don't thrash table loads. Indices into act_info.json must be preserved, so
    the shadowing single-function sets are emptied in place, not removed."""
    import concourse.bacc as _bacc

    orig = _bacc.get_activation_tables
    if getattr(_bacc, "_mla_act_patch", False):
        return
    _bacc._mla_act_patch = True

    def patched(arch):
        d = dict(orig(arch))
        if "natural_log_exp_and_others" in d:
            for name in ("exp_and_others", "natural_log", "exp_and_friends"):
                if name in d:
                    d[name] = set()
        return d

    _bacc.get_activation_tables = patched


def build_program():
    _patch_act_tables()
    nc = bacc.Bacc("TRN2", target_bir_lowering=False, debug=False, num_devices=NCORES)
    xt_d = nc.dram_tensor("xt", [D, B * T], bf16, kind="ExternalInput")
    w1_d = nc.dram_tensor("w1", [D, W1N], bf16, kind="ExternalInput")
    wb_d = nc.dram_tensor("wb", [KVR, HPC * (NOPE + VD)], bf16, kind="ExternalInput")
    wo_d = nc.dram_tensor("wo", [H * VD, D], bf16, kind="ExternalInput")
    cos_d = nc.dram_tensor("cos", [128, T], bf16, kind="ExternalInput")
    sin_d = nc.dram_tensor("sin", [128, T], bf16, kind="ExternalInput")
    msk_d = nc.dram_tensor("msk", [128, 4 * BLK], bf16, kind="ExternalInput")
    out_d = nc.dram_tensor("out", [B, T // NCORES, D], bf16, kind="ExternalOutput")

    RG = [list(range(NCORES))]

    with tile.TileContext(nc) as tc:
        with (
            tc.tile_pool(name="dram", bufs=1, space="DRAM") as dram,
            tc.tile_pool(name="const", bufs=1) as const,
            tc.tile_pool(name="wpool", bufs=1) as wpool,
            tc.tile_pool(name="kvpool", bufs=1) as kvpool,
            tc.tile_pool(name="work", bufs=1) as work,
            tc.tile_pool(name="wop", bufs=1) as wop,
            tc.tile_pool(name="ps", bufs=1, space="PSUM") as ps,
        ):
            y_in = [
                dram.tile([NCORES, HPC * VD, 256], bf16, name=f"y_in{b}")
                for b in range(B)
            ]
            y_out = [
                dram.tile([NCORES, HPC * VD, 256], bf16, name=f"y_out{b}")
                for b in range(B)
            ]
            ag_in = [
                [dram.tile([KVS, BLK], bf16, name=f"ag_in{b}_{q}") for q in range(4)]
                for b in range(B)
            ]
            ag_out = [
                [
                    dram.tile(
                        [KVC, BLK], bf16, name=f"ag_out{b}_{q}", addr_space="Shared"
                    )
                    for q in range(4)
                ]
                for b in range(B)
            ]

            agd_in = dram.tile([1, 64], bf16, name="agd_in")
            agd_out = dram.tile([8, 64], bf16, name="agd_out", addr_space="Shared")

            ones_f = const.tile([128, 1], f32, tag="ones_f")
            nc.gpsimd.memset(ones_f[:], 1.0)
            ones_r = const.tile([128, 1], f32r, tag="ones_r")
            nc.vector.tensor_copy(ones_r[:], ones_f[:])
            ones_b = const.tile([128, 1], bf16, tag="ones_b")
            nc.vector.tensor_copy(ones_b[:], ones_f[:])
            onesrow_f = const.tile([1, 128], f32, tag="onesrow_f")
            nc.gpsimd.memset(onesrow_f[:], 1.0)
            onesrow_b = const.tile([1, 128], bf16, tag="onesrow_b")
            nc.vector.tensor_copy(onesrow_b[:], onesrow_f[:])
            eps = const.tile([1, 1], f32, tag="eps")
            nc.gpsimd.memset(eps[:], 1e-6)

            dummy_sb = const.tile([1, 64], bf16, tag="dummy")
            nc.gpsimd.memset(dummy_sb[:], 0.0)
            nc.sync.dma_start(agd_in[:], dummy_sb[:])
            nc.gpsimd.collective_compute(
                "AllGather",
                mybir.AluOpType.bypass,
                replica_groups=RG,
                ins=[agd_in.opt()],
                outs=[agd_out.opt()],
            )

            w1_sb = wpool.tile([128, 16, W1N], bf16, tag="w1")
            nc.sync.dma_start(w1_sb[:], w1_d[:].rearrange("(kc p) m -> p kc m", p=128))
            wb_sb = wpool.tile([128, 4, 512], bf16, tag="wb")
            nc.sync.dma_start(wb_sb[:], wb_d[:].rearrange("(kc p) m -> p kc m", p=128))
            cs_sb = wpool.tile([128, T], bf16, tag="cs")
            nc.sync.dma_start(cs_sb[:], cos_d[:])
            sn_sb = wpool.tile([128, T], bf16, tag="sn")
            nc.sync.dma_start(sn_sb[:], sin_d[:])
            msk_sb = wpool.tile([128, 4 * BLK], bf16, tag="msk")

            def load_msk():
                nc.sync.dma_start(msk_sb[:], msk_d[:])
            wo_sb = wpool.tile([128, 16, D], bf16, tag="wo")

            def load_wo():
                nc.sync.dma_start(
                    wo_sb[:], wo_d[:].rearrange("(kc p) m -> p kc m", p=128)
                )

            # per-batch persistent kv staging (explicit per-batch tiles so
            # batch 1's writes never wait on batch 0's attention reads)
            def alloc_kv(b):
                knope = [
                    kvpool.tile([NOPE, T], bf16, name=f"knope{b}_{h}")
                    for h in range(2)
                ]
                vnat = [
                    kvpool.tile([128, 16, VD], bf16, name=f"vnat{b}_{h}")
                    for h in range(2)
                ]
                krope = kvpool.tile([ROPE, T], bf16, name=f"krope{b}")
                return knope, vnat, krope

            def stage_a1(b, qc):
                """x load (xbar-transposed) + fused q/kvc projection + q rope + AG."""
                row0 = b * T + qc * BLK
                tok = slice(qc * BLK, (qc + 1) * BLK)

                pp = [
                    ps.tile([128, BLK], f32, tag="proj", bufs=4, name="projp")
                    for _ in range(4)
                ]
                for kc in range(16):
                    xTc = work.tile([128, BLK], bf16, tag="xTc", bufs=8)
                    nc.sync.dma_start(
                        xTc[:],
                        xt_d[kc * 128 : (kc + 1) * 128, row0 : row0 + BLK],
                    )
                    for mc in range(4):
                        m0 = mc * 128
                        m1 = min(m0 + 128, W1N)
                        nc.tensor.matmul(
                            pp[mc][: m1 - m0, :],
                            w1_sb[:, kc, m0:m1],
                            xTc[:],
                            start=(kc == 0),
                            stop=(kc == 15),
                        )

                # my kvc slice -> dram -> allgather (first, to trigger AG asap)
                kvcm = work.tile([KVS, BLK], bf16, tag="kvcm", bufs=2)
                nc.vector.tensor_copy(kvcm[:], pp[3][:KVS, :])
                nc.sync.dma_start(ag_in[b][qc][:], kvcm[:])
                nc.gpsimd.collective_compute(
                    "AllGather",
                    mybir.AluOpType.bypass,
                    replica_groups=RG,
                    ins=[ag_in[b][qc].opt()],
                    outs=[ag_out[b][qc].opt()],
                )

                # q: nope chunks straight, rope chunk roped
                qfT = work.tile([128, 4, BLK], bf16, tag="qfT", bufs=4)
                nc.vector.tensor_copy(qfT[:, 0, :], pp[0][:])
                nc.vector.tensor_copy(qfT[:, 1, :], pp[1][:])
                rot = work.tile([128, BLK], bf16, tag="rot", bufs=2)
                pq = pp[2]
                for hh in range(2):
                    r0 = hh * 64
                    nc.vector.tensor_scalar_mul(
                        rot[r0 : r0 + 32, :], pq[r0 + 32 : r0 + 64, :], -1.0
                    )
                    nc.vector.tensor_copy(
                        rot[r0 + 32 : r0 + 64, :], pq[r0 : r0 + 32, :]
                    )
                nc.vector.tensor_mul(out=qfT[:, 2, :], in0=pq[:], in1=cs_sb[:, tok])
                nc.vector.tensor_mul(out=rot[:], in0=rot[:], in1=sn_sb[:, tok])
                nc.vector.tensor_add(out=qfT[:, 2, :], in0=qfT[:, 2, :], in1=rot[:])
                # h1 roped rows 64:128 -> chunk 3 rows 0:64 (32-wide moves)
                nc.vector.tensor_copy(qfT[0:32, 3, :], qfT[64:96, 2, :])
                nc.vector.tensor_copy(qfT[32:64, 3, :], qfT[96:128, 2, :])
                return qfT

            def stage_a2(b, qc, kv):
                """post-AG: rms norm, kv up-projection, k rope."""
                knope, vnat, krope = kv
                tok = slice(qc * BLK, (qc + 1) * BLK)
                latent = work.tile([128, 4, BLK], bf16, tag="latent", bufs=2)
                nc.scalar.dma_start(
                    latent[:],
                    ag_out[b][qc][:KVR, :].rearrange("(kc p) t -> p kc t", p=128),
                )
                kraw = work.tile([ROPE, BLK], bf16, tag="kraw", bufs=2)
                nc.scalar.dma_start(kraw[:], ag_out[b][qc][KVR:, :])

                # sum of squares over latent dims (ACT square + PE ones-mm)
                ssq = ps.tile([1, BLK], f32, tag="xps", bufs=1, name="ssq")
                for i in range(4):
                    sqc = work.tile([128, BLK], bf16, tag="sqc", bufs=1)
                    nc.scalar.activation(sqc[:], latent[:, i, :], SQUARE)
                    nc.tensor.matmul(
                        ssq[:], ones_b[:], sqc[:], start=(i == 0), stop=(i == 3)
                    )
                # rms scale: 1/sqrt(ssq/512+eps) = exp(-0.5*ln(.))
                lnrow = work.tile([1, BLK], f32, tag="lnrow", bufs=2)
                nc.scalar.activation(lnrow[:], ssq[:], LN, bias=eps[:], scale=1.0 / KVR)
                invrow = work.tile([1, BLK], bf16, tag="invrow", bufs=2)
                nc.scalar.activation(invrow[:], lnrow[:], EXP, scale=-0.5)
                invbc_ps = ps.tile([128, BLK], f32, tag="xps", bufs=1, name="invbc_ps")
                nc.tensor.matmul(invbc_ps[:], onesrow_b[:], invrow[:])
                invbc = work.tile([128, BLK], bf16, tag="invbc", bufs=2)
                nc.vector.tensor_copy(invbc[:], invbc_ps[:])

                # k rope from gathered raw rows
                rot = work.tile([128, BLK], bf16, tag="rot", bufs=2)
                kr = krope[:, tok]
                nc.vector.tensor_scalar_mul(rot[0:32, :], kraw[32:64, :], -1.0)
                nc.vector.tensor_copy(rot[32:64, :], kraw[0:32, :])
                nc.vector.tensor_mul(out=kr, in0=kraw[:], in1=cs_sb[0:64, tok])
                nc.vector.tensor_mul(
                    out=rot[0:64, :], in0=rot[0:64, :], in1=sn_sb[0:64, tok]
                )
                nc.vector.tensor_add(out=kr, in0=kr, in1=rot[0:64, :])

                # kv up-projection + normalize; v transposed to natural
                for mc in range(4):  # [h0 nope, h0 v, h1 nope, h1 v]
                    h = mc // 2
                    pkv = ps.tile([128, BLK], f32, tag="proj", bufs=4)
                    for kc in range(4):
                        nc.tensor.matmul(
                            pkv[:],
                            wb_sb[:, kc, mc * 128 : (mc + 1) * 128],
                            latent[:, kc, :],
                            start=(kc == 0),
                            stop=(kc == 3),
                        )
                    if mc % 2 == 0:
                        nc.vector.tensor_mul(
                            out=knope[h][:, tok], in0=pkv[:], in1=invbc[:]
                        )
                    else:
                        vuT = work.tile([128, BLK], bf16, tag="vuT", bufs=2)
                        nc.vector.tensor_mul(out=vuT[:], in0=pkv[:], in1=invbc[:])
                        nc.sync.dma_start_transpose(
                            vnat[h][:, qc * 4 : qc * 4 + 4, :], vuT[:]
                        )

            def stage_bh(b, qc, h, qfT, kv):
                """causal attention for one q-chunk, one head.

                Software-pipelined with lookahead 2: the yacc MM for kt
                issues after the score MMs for kt+2, so the PE never waits
                on the exp/mask chain.
                """
                knope, vnat, krope = kv
                n_kt = 4 * (qc + 1)
                yacc = ps.tile([VD, BLK], f32, tag="yacc", bufs=1)
                acc = work.tile([128, BLK], f32r, tag="acc", bufs=2)
                qrope = qfT[0:64, 2 + h, :]
                pend = []
                for kt in range(n_kt):
                    ks = slice(kt * 128, (kt + 1) * 128)
                    st = ps.tile([128, BLK], f32, tag="st", bufs=2)
                    nc.tensor.matmul(
                        st[:], knope[h][:, ks], qfT[:, h, :],
                        start=True, stop=False,
                    )
                    nc.tensor.matmul(
                        st[:], krope[:, ks], qrope, start=False, stop=True
                    )
                    if len(pend) == 2:
                        pe_est, pk = pend.pop(0)
                        nc.tensor.matmul(
                            yacc[:], vnat[h][:, pk, :], pe_est[:],
                            start=(pk == 0), stop=False,
                        )
                    est = work.tile([128, BLK], bf16, tag="est", bufs=3)
                    nc.scalar.activation(est[:], st[:], EXP)
                    j = kt - 4 * qc
                    if j >= 0:
                        nc.vector.tensor_mul(
                            out=est[:], in0=est[:],
                            in1=msk_sb[:, j * BLK : (j + 1) * BLK],
                        )
                    if kt == 0:
                        nc.gpsimd.tensor_copy(acc[:], est[:])
                    else:
                        nc.gpsimd.tensor_add(out=acc[:], in0=acc[:], in1=est[:])
                    pend.append((est, kt))
                for pe_est, pk in pend:
                    nc.tensor.matmul(
                        yacc[:], vnat[h][:, pk, :], pe_est[:],
                        start=(pk == 0), stop=(pk == n_kt - 1),
                    )

                sums = ps.tile([1, BLK], f32, tag="xps", bufs=1, name="sums")
                nc.tensor.matmul(sums[:], ones_r[:], acc[:])
                lnr = work.tile([1, BLK], f32, tag="lnrow", bufs=2)
                nc.scalar.activation(lnr[:], sums[:], LN)
                sinvrow = work.tile([1, BLK], bf16, tag="invrow", bufs=2)
                nc.scalar.activation(sinvrow[:], lnr[:], EXP, scale=-1.0)
                sbc_ps = ps.tile([128, BLK], f32, tag="xps", bufs=1, name="sbc_ps")
                nc.tensor.matmul(sbc_ps[:], onesrow_b[:], sinvrow[:])
                sinv = work.tile([128, BLK], bf16, tag="sinv", bufs=2)
                nc.vector.tensor_copy(sinv[:], sbc_ps[:])
                ysb = work.tile([VD, BLK], bf16, tag="ysb", bufs=2)
                nc.vector.tensor_mul(out=ysb[:], in0=yacc[:], in1=sinv[:])
                for jj in range(2):
                    nc.sync.dma_start(
                        y_in[b][qc * 2 + jj, h * VD : (h + 1) * VD, :],
                        ysb[:, jj * 256 : (jj + 1) * 256],
                    )

            def emit_a2a(b):
                nc.gpsimd.collective_compute(
                    "AllToAll",
                    mybir.AluOpType.bypass,
                    replica_groups=RG,
                    ins=[y_in[b].opt()],
                    outs=[y_out[b].opt()],
                )

            def emit_wo(b, spread=False):
                """wo projection for this batch's gathered token slice.

                spread=True (final batch, all PSUM banks free): kc-outer
                over 8 concurrent bank accumulators, so wo MMs start as the
                first AllToAll chunk lands instead of after all 16.
                """
                a2a = wop.tile([128, 16, 256], bf16, tag="a2a", bufs=1, name="a2a")
                for kc in range(16):
                    nc.gpsimd.dma_start(
                        a2a[:, kc, :],
                        y_out[b][kc // 2, (kc % 2) * 128 : (kc % 2) * 128 + 128, :],
                    )
                if spread:
                    tags = ["proj", "proj", "proj", "proj", "st", "st", "xps", "yacc"]
                    bufn = [4, 4, 4, 4, 2, 2, 1, 1]
                    pouts = [
                        ps.tile([128, 512], f32, tag=tg, bufs=bu, name="pout")
                        for tg, bu in zip(tags, bufn)
                    ]
                    for kc in range(16):
                        for g, pout in enumerate(pouts):
                            tt, n = g % 2, g // 2
                            nc.tensor.matmul(
                                pout[:],
                                a2a[:, kc, tt * 128 : (tt + 1) * 128],
                                wo_sb[:, kc, n * 512 : (n + 1) * 512],
                                start=(kc == 0),
                                stop=(kc == 15),
                            )
                    for g, pout in enumerate(pouts):
                        tt, n = g % 2, g // 2
                        osb = wop.tile([128, 512], bf16, tag="osb", bufs=2)
                        nc.vector.tensor_copy(osb[:], pout[:])
                        nc.sync.dma_start(
                            out_d[
                                b, tt * 128 : (tt + 1) * 128, n * 512 : (n + 1) * 512
                            ],
                            osb[:],
                        )
                    return
                for n in range(4):
                    for tt in range(2):
                        pout = ps.tile([128, 512], f32, tag="yacc", bufs=1, name="pout")
                        for kc in range(16):
                            nc.tensor.matmul(
                                pout[:],
                                a2a[:, kc, tt * 128 : (tt + 1) * 128],
                                wo_sb[:, kc, n * 512 : (n + 1) * 512],
                                start=(kc == 0),
                                stop=(kc == 15),
                            )
                        osb = wop.tile([128, 512], bf16, tag="osb", bufs=2)
                        nc.vector.tensor_copy(osb[:], pout[:])
                        nc.sync.dma_start(
                            out_d[
                                b, tt * 128 : (tt + 1) * 128, n * 512 : (n + 1) * 512
                            ],
                            osb[:],
                        )

            # ---- software-pipelined schedule ----
            kv0 = alloc_kv(0)
            kv1 = alloc_kv(1)
            q00 = stage_a1(0, 0)
            q01 = stage_a1(0, 1)
            load_msk()
            q02 = stage_a1(0, 2)
            q03 = stage_a1(0, 3)
            load_wo()
            stage_a2(0, 0, kv0)
            stage_bh(0, 0, 0, q00, kv0)
            stage_a2(0, 1, kv0)
            stage_bh(0, 0, 1, q00, kv0)
            stage_bh(0, 1, 0, q01, kv0)
            stage_a2(0, 2, kv0)
            stage_bh(0, 1, 1, q01, kv0)
            stage_bh(0, 2, 0, q02, kv0)
            stage_a2(0, 3, kv0)
            stage_bh(0, 2, 1, q02, kv0)
            stage_bh(0, 3, 0, q03, kv0)
            stage_bh(0, 3, 1, q03, kv0)
            emit_a2a(0)
            q10 = stage_a1(1, 0)
            q11 = stage_a1(1, 1)
            q12 = stage_a1(1, 2)
            q13 = stage_a1(1, 3)
            stage_a2(1, 0, kv1)
            emit_wo(0)
            stage_bh(1, 0, 0, q10, kv1)
            stage_a2(1, 1, kv1)
            stage_bh(1, 0, 1, q10, kv1)
            stage_bh(1, 1, 0, q11, kv1)
            stage_a2(1, 2, kv1)
            stage_bh(1, 1, 1, q11, kv1)
            stage_bh(1, 2, 0, q12, kv1)
            stage_a2(1, 3, kv1)
            stage_bh(1, 2, 1, q12, kv1)
            stage_bh(1, 3, 0, q13, kv1)
            stage_bh(1, 3, 1, q13, kv1)
            emit_a2a(1)
            emit_wo(1, spread=True)

    nc.compile()
    return nc


def host_prep(x, wq, wkv_a, wkv_b, wo, kv_norm_w):
    bf = ml_dtypes.bfloat16
    scale = np.float32(QKD ** -0.5)
    inv = (1.0 / (10000.0 ** (np.arange(0, ROPE, 2, dtype=np.float32) / ROPE))).astype(
        np.float32
    )
    f = np.outer(np.arange(T, dtype=np.float32), inv)
    cos32 = np.cos(f).T.astype(np.float32)
    sin32 = np.sin(f).T.astype(np.float32)
    cos128 = np.ascontiguousarray(np.concatenate([cos32] * 4, 0)).astype(bf)
    sin128 = np.ascontiguousarray(np.concatenate([sin32] * 4, 0)).astype(bf)
    wkv_bw = (wkv_b * kv_norm_w[:, None]).astype(np.float32)
    xt = np.ascontiguousarray(x.reshape(B * T, D).T).astype(bf)
    wo_c = np.ascontiguousarray(wo).astype(bf)
    wq_r = wq.reshape(D, H, QKD)

    kk = np.arange(128)[:, None]
    qq = np.arange(BLK)[None, :]
    msk = np.concatenate(
        [(qq >= kk + j * 128).astype(np.float32) for j in range(4)], axis=1
    ).astype(bf)

    in_maps = []
    for c in range(NCORES):
        h0 = HPC * c
        w1 = np.concatenate(
            [
                wq_r[:, h0, :NOPE] * scale,
                wq_r[:, h0 + 1, :NOPE] * scale,
                wq_r[:, h0, NOPE:] * scale,
                wq_r[:, h0 + 1, NOPE:] * scale,
                wkv_a[:, c * KVS : (c + 1) * KVS],
            ],
            axis=1,
        ).astype(bf)
        wb = np.ascontiguousarray(
            wkv_bw[:, h0 * (NOPE + VD) : (h0 + 2) * (NOPE + VD)]
        ).astype(bf)
        in_maps.append(
            {
                "xt": xt,
                "w1": np.ascontiguousarray(w1),
                "wb": wb,
                "wo": wo_c,
                "cos": cos128,
                "sin": sin128,
                "msk": msk,
            }
        )
    return in_maps


_NC = None


def kernel(x, wq, wkv_a, wkv_b, wo, kv_norm_w, _trace=False):
    global _NC
    if _NC is None:
        _NC = build_program()
    in_maps = host_prep(
        np.asarray(x, np.float32),
        np.asarray(wq, np.float32),
        np.asarray(wkv_a, np.float32),
        np.asarray(wkv_b, np.float32),
        np.asarray(wo, np.float32),
        np.asarray(kv_norm_w, np.float32),
    )
    res = run_bass_kernel_spmd(_NC, in_maps, list(range(NCORES)), trace=_trace)
    out = np.empty((B, T, D), np.float32)
    cw = T // NCORES
    for c in range(NCORES):
        oc = res.results[c]["out"].astype(np.float32)  # (B, 256, D)
        for b in range(B):
            out[b, c * cw : (c + 1) * cw, :] = oc[b]
    kernel.last_results = res
    return out


# revision 26
# speedup vs baseline: 1.0307x; 1.0307x over previous
"""Multi-Head Latent Attention (MLA) forward on 8 Trainium2 NeuronCores.

Sharding: tensor-parallel over heads (16 heads -> 2 per core). Each core:
  - loads x pre-transposed (host ships x.T in bf16),
  - computes q projections for its heads plus a 1/8 column slice of the
    latent-kv encoding; per-block AllGather assembles the full latent
    (hidden behind compute),
  - RMS-norms the latent (norm weight folded into wkv_b on host, per-token
    scale applied after the up-projection),
  - applies RoPE with host-precomputed cos/sin tables,
  - runs causal attention for its 2 heads in transposed-score layout
    (scores St[k, q]; softmax without max subtraction - scores are O(1)),
  - per-batch AllToAll exchanges head outputs so each core holds all
    features for a token slice, then computes that slice of the wo
    projection from a SBUF-resident bf16 copy of wo.
Batch 1's projection work is interleaved into batch 0's attention phase and
wo(0) into batch 1's phase so the PE never idles across the collectives.
Output slices are disjoint; the host just concatenates them.
"""
import sys

if "/opt/trn_rl_repo" not in sys.path:
    sys.path.insert(0, "/opt/trn_rl_repo")

import numpy as np
import ml_dtypes
import concourse.bacc as bacc
import concourse.mybir as mybir
from concourse import tile
from concourse.bass_utils import run_bass_kernel_spmd

H, NOPE, ROPE, VD, KVR, QKD = 16, 128, 64, 128, 512, 192
B, T, D = 2, 2048, 2048
NCORES, HPC, BLK = 8, 2, 512
KVC = KVR + ROPE  # 576 latent+rope columns
KVS = KVC // NCORES  # 72-column slice per core
W1N = HPC * QKD + KVS  # 456 projection columns per core
f32 = mybir.dt.float32
f32r = mybir.dt.float32r
bf16 = mybir.dt.bfloat16
EXP = mybir.ActivationFunctionType.Exp
LN = mybir.ActivationFunctionType.Ln
SQUARE = mybir.ActivationFunctionType.Square


def r32(ap):
    return ap.bitcast(f32r)


def _patch_act_tables():
    """Make the act-table-load pass serve Exp/Ln/Square from the one set that
    contains them all (natural_log_exp_and_others), so interleaved activations
    don't thrash table loads. Indices into act_info.json must be preserved, so
    the shadowing single-function sets are emptied in place, not removed."""
    import concourse.bacc as _bacc

    orig = _bacc.get_activation_tables
    if getattr(_bacc, "_mla_act_patch", False):
        return
    _bacc._mla_act_patch = True

    def patched(arch):
        d = dict(orig(arch))
        if "natural_log_exp_and_others" in d:
            for name in ("exp_and_others", "natural_log", "exp_and_friends"):
                if name in d:
                    d[name] = set()
        return d

    _bacc.get_activation_tables = patched


def build_program():
    _patch_act_tables()
    nc = bacc.Bacc("TRN2", target_bir_lowering=False, debug=False, num_devices=NCORES)
    xt_d = nc.dram_tensor("xt", [D, B * T], bf16, kind="ExternalInput")
    w1_d = nc.dram_tensor("w1", [D, W1N], bf16, kind="ExternalInput")
    wb_d = nc.dram_tensor("wb", [KVR, HPC * (NOPE + VD)], bf16, kind="ExternalInput")
    wo_d = nc.dram_tensor("wo", [H * VD, D], bf16, kind="ExternalInput")
    cos_d = nc.dram_tensor("cos", [128, T], bf16, kind="ExternalInput")
    sin_d = nc.dram_tensor("sin", [128, T], bf16, kind="ExternalInput")
    msk_d = nc.dram_tensor("msk", [128, 4 * BLK], bf16, kind="ExternalInput")
    out_d = nc.dram_tensor("out", [B, T // NCORES, D], bf16, kind="ExternalOutput")

    RG = [list(range(NCORES))]

    with tile.TileContext(nc) as tc:
        with (
            tc.tile_pool(name="dram", bufs=1, space="DRAM") as dram,
            tc.tile_pool(name="const", bufs=1) as const,
            tc.tile_pool(name="wpool", bufs=1) as wpool,
            tc.tile_pool(name="kvpool", bufs=1) as kvpool,
            tc.tile_pool(name="work", bufs=1) as work,
            tc.tile_pool(name="wop", bufs=1) as wop,
            tc.tile_pool(name="ps", bufs=1, space="PSUM") as ps,
        ):
            y_in = [
                dram.tile([NCORES, HPC * VD, 256], bf16, name=f"y_in{b}")
                for b in range(B)
            ]
            y_out = [
                dram.tile([NCORES, HPC * VD, 256], bf16, name=f"y_out{b}")
                for b in range(B)
            ]
            ag_in = [
                [dram.tile([KVS, BLK], bf16, name=f"ag_in{b}_{q}") for q in range(4)]
                for b in range(B)
            ]
            ag_out = [
                [
                    dram.tile(
                        [KVC, BLK], bf16, name=f"ag_out{b}_{q}", addr_space="Shared"
                    )
                    for q in range(4)
                ]
                for b in range(B)
            ]

            agd_in = dram.tile([1, 64], bf16, name="agd_in")
            agd_out = dram.tile([8, 64], bf16, name="agd_out", addr_space="Shared")

            ones_f = const.tile([128, 1], f32, tag="ones_f")
            nc.gpsimd.memset(ones_f[:], 1.0)
            ones_r = const.tile([128, 1], f32r, tag="ones_r")
            nc.vector.tensor_copy(ones_r[:], ones_f[:])
            ones_b = const.tile([128, 1], bf16, tag="ones_b")
            nc.vector.tensor_copy(ones_b[:], ones_f[:])
            onesrow_f = const.tile([1, 128], f32, tag="onesrow_f")
            nc.gpsimd.memset(onesrow_f[:], 1.0)
            onesrow_b = const.tile([1, 128], bf16, tag="onesrow_b")
            nc.vector.tensor_copy(onesrow_b[:], onesrow_f[:])
            eps = const.tile([1, 1], f32, tag="eps")
            nc.gpsimd.memset(eps[:], 1e-6)

            dummy_sb = const.tile([1, 64], bf16, tag="dummy")
            nc.gpsimd.memset(dummy_sb[:], 0.0)
            nc.sync.dma_start(agd_in[:], dummy_sb[:])
            nc.gpsimd.collective_compute(
                "AllGather",
                mybir.AluOpType.bypass,
                replica_groups=RG,
                ins=[agd_in.opt()],
                outs=[agd_out.opt()],
            )

            w1_sb = wpool.tile([128, 16, W1N], bf16, tag="w1")
            nc.sync.dma_start(w1_sb[:], w1_d[:].rearrange("(kc p) m -> p kc m", p=128))
            wb_sb = wpool.tile([128, 4, 512], bf16, tag="wb")
            nc.sync.dma_start(wb_sb[:], wb_d[:].rearrange("(kc p) m -> p kc m", p=128))
            cs_sb = wpool.tile([128, T], bf16, tag="cs")
            nc.sync.dma_start(cs_sb[:], cos_d[:])
            sn_sb = wpool.tile([128, T], bf16, tag="sn")
            nc.sync.dma_start(sn_sb[:], sin_d[:])
            msk_sb = wpool.tile([128, 4 * BLK], bf16, tag="msk")

            def load_msk():
                nc.sync.dma_start(msk_sb[:], msk_d[:])
            wo_sb = wpool.tile([128, 16, D], bf16, tag="wo")

            def load_wo():
                nc.sync.dma_start(
                    wo_sb[:], wo_d[:].rearrange("(kc p) m -> p kc m", p=128)
                )

            # per-batch persistent kv staging (explicit per-batch tiles so
            # batch 1's writes never wait on batch 0's attention reads)
            def alloc_kv(b):
                knope = [
                    kvpool.tile([NOPE, T], bf16, name=f"knope{b}_{h}")
                    for h in range(2)
                ]
                vnat = [
                    kvpool.tile([128, 16, VD], bf16, name=f"vnat{b}_{h}")
                    for h in range(2)
                ]
                krope = kvpool.tile([ROPE, T], bf16, name=f"krope{b}")
                return knope, vnat, krope

            def stage_a1(b, qc):
                """x load (xbar-transposed) + fused q/kvc projection + q rope + AG."""
                row0 = b * T + qc * BLK
                tok = slice(qc * BLK, (qc + 1) * BLK)

                pp = [
                    ps.tile([128, BLK], f32, tag="proj", bufs=4, name="projp")
                    for _ in range(4)
                ]
                for kc in range(16):
                    xTc = work.tile([128, BLK], bf16, tag="xTc", bufs=8)
                    nc.sync.dma_start(
                        xTc[:],
                        xt_d[kc * 128 : (kc + 1) * 128, row0 : row0 + BLK],
                    )
                    for mc in range(4):
                        m0 = mc * 128
                        m1 = min(m0 + 128, W1N)
                        nc.tensor.matmul(
                            pp[mc][: m1 - m0, :],
                            w1_sb[:, kc, m0:m1],
                            xTc[:],
                            start=(kc == 0),
                            stop=(kc == 15),
                        )

                # my kvc slice -> dram -> allgather (first, to trigger AG asap)
                kvcm = work.tile([KVS, BLK], bf16, tag="kvcm", bufs=2)
                nc.vector.tensor_copy(kvcm[:], pp[3][:KVS, :])
                nc.sync.dma_start(ag_in[b][qc][:], kvcm[:])
                nc.gpsimd.collective_compute(
                    "AllGather",
                    mybir.AluOpType.bypass,
                    replica_groups=RG,
                    ins=[ag_in[b][qc].opt()],
                    outs=[ag_out[b][qc].opt()],
                )

                # q: nope chunks straight, rope chunk roped
                qfT = work.tile([128, 4, BLK], bf16, tag="qfT", bufs=4)
                nc.vector.tensor_copy(qfT[:, 0, :], pp[0][:])
                nc.vector.tensor_copy(qfT[:, 1, :], pp[1][:])
                rot = work.tile([128, BLK], bf16, tag="rot", bufs=2)
                pq = pp[2]
                for hh in range(2):
                    r0 = hh * 64
                    nc.vector.tensor_scalar_mul(
                        rot[r0 : r0 + 32, :], pq[r0 + 32 : r0 + 64, :], -1.0
                    )
                    nc.vector.tensor_copy(
                        rot[r0 + 32 : r0 + 64, :], pq[r0 : r0 + 32, :]
                    )
                nc.vector.tensor_mul(out=qfT[:, 2, :], in0=pq[:], in1=cs_sb[:, tok])
                nc.vector.tensor_mul(out=rot[:], in0=rot[:], in1=sn_sb[:, tok])
                nc.vector.tensor_add(out=qfT[:, 2, :], in0=qfT[:, 2, :], in1=rot[:])
                # h1 roped rows 64:128 -> chunk 3 rows 0:64 (32-wide moves)
                nc.vector.tensor_copy(qfT[0:32, 3, :], qfT[64:96, 2, :])
                nc.vector.tensor_copy(qfT[32:64, 3, :], qfT[96:128, 2, :])
                return qfT

            def stage_a2(b, qc, kv):
                """post-AG: rms norm, kv up-projection, k rope."""
                knope, vnat, krope = kv
                tok = slice(qc * BLK, (qc + 1) * BLK)
                latent = work.tile([128, 4, BLK], bf16, tag="latent", bufs=2)
                nc.scalar.dma_start(
                    latent[:],
                    ag_out[b][qc][:KVR, :].rearrange("(kc p) t -> p kc t", p=128),
                )
                kraw = work.tile([ROPE, BLK], bf16, tag="kraw", bufs=2)
                nc.scalar.dma_start(kraw[:], ag_out[b][qc][KVR:, :])

                # sum of squares over latent dims (ACT square + PE ones-mm)
                ssq = ps.tile([1, BLK], f32, tag="xps", bufs=1, name="ssq")
                for i in range(4):
                    sqc = work.tile([128, BLK], bf16, tag="sqc", bufs=1)
                    nc.scalar.activation(sqc[:], latent[:, i, :], SQUARE)
                    nc.tensor.matmul(
                        ssq[:], ones_b[:], sqc[:], start=(i == 0), stop=(i == 3)
                    )
                # rms scale: 1/sqrt(ssq/512+eps) = exp(-0.5*ln(.))
                lnrow = work.tile([1, BLK], f32, tag="lnrow", bufs=2)
                nc.scalar.activation(lnrow[:], ssq[:], LN, bias=eps[:], scale=1.0 / KVR)
                invrow = work.tile([1, BLK], bf16, tag="invrow", bufs=2)
                nc.scalar.activation(invrow[:], lnrow[:], EXP, scale=-0.5)
                invbc_ps = ps.tile([128, BLK], f32, tag="xps", bufs=1, name="invbc_ps")
                nc.tensor.matmul(invbc_ps[:], onesrow_b[:], invrow[:])
                invbc = work.tile([128, BLK], bf16, tag="invbc", bufs=2)
                nc.vector.tensor_copy(invbc[:], invbc_ps[:])

                # k rope from gathered raw rows
                rot = work.tile([128, BLK], bf16, tag="rot", bufs=2)
                kr = krope[:, tok]
                nc.vector.tensor_scalar_mul(rot[0:32, :], kraw[32:64, :], -1.0)
                nc.vector.tensor_copy(rot[32:64, :], kraw[0:32, :])
                nc.vector.tensor_mul(out=kr, in0=kraw[:], in1=cs_sb[0:64, tok])
                nc.vector.tensor_mul(
                    out=rot[0:64, :], in0=rot[0:64, :], in1=sn_sb[0:64, tok]
                )
                nc.vector.tensor_add(out=kr, in0=kr, in1=rot[0:64, :])

                # kv up-projection + normalize; v transposed to natural
                for mc in range(4):  # [h0 nope, h0 v, h1 nope, h1 v]
                    h = mc // 2
                    pkv = ps.tile([128, BLK], f32, tag="proj", bufs=4)
                    for kc in range(4):
                        nc.tensor.matmul(
                            pkv[:],
                            wb_sb[:, kc, mc * 128 : (mc + 1) * 128],
                            latent[:, kc, :],
                            start=(kc == 0),
                            stop=(kc == 3),
                        )
                    if mc % 2 == 0:
                        nc.vector.tensor_mul(
                            out=knope[h][:, tok], in0=pkv[:], in1=invbc[:]
                        )
                    else:
                        vuT = work.tile([128, BLK], bf16, tag="vuT", bufs=2)
                        nc.vector.tensor_mul(out=vuT[:], in0=pkv[:], in1=invbc[:])
                        nc.sync.dma_start_transpose(
                            vnat[h][:, qc * 4 : qc * 4 + 4, :], vuT[:]
                        )

            def stage_bh(b, qc, h, qfT, kv):
                """causal attention for one q-chunk, one head.

                Software-pipelined with lookahead 2: the yacc MM for kt
                issues after the score MMs for kt+2, so the PE never waits
                on the exp/mask chain.
                """
                knope, vnat, krope = kv
                n_kt = 4 * (qc + 1)
                yacc = ps.tile([VD, BLK], f32, tag="yacc", bufs=1)
                acc = work.tile([128, BLK], f32r, tag="acc", bufs=2)
                qrope = qfT[0:64, 2 + h, :]
                pend = []
                for kt in range(n_kt):
                    ks = slice(kt * 128, (kt + 1) * 128)
                    st = ps.tile([128, BLK], f32, tag="st", bufs=2)
                    nc.tensor.matmul(
                        st[:], knope[h][:, ks], qfT[:, h, :],
                        start=True, stop=False,
                    )
                    nc.tensor.matmul(
                        st[:], krope[:, ks], qrope, start=False, stop=True
                    )
                    if len(pend) == 2:
                        pe_est, pk = pend.pop(0)
                        nc.tensor.matmul(
                            yacc[:], vnat[h][:, pk, :], pe_est[:],
                            start=(pk == 0), stop=False,
                        )
                    est = work.tile([128, BLK], bf16, tag="est", bufs=4)
                    nc.scalar.activation(est[:], st[:], EXP)
                    j = kt - 4 * qc
                    if j >= 0:
                        nc.vector.tensor_mul(
                            out=est[:], in0=est[:],
                            in1=msk_sb[:, j * BLK : (j + 1) * BLK],
                        )
                    if kt == 0:
                        nc.gpsimd.tensor_copy(acc[:], est[:])
                    else:
                        nc.gpsimd.tensor_add(out=acc[:], in0=acc[:], in1=est[:])
                    pend.append((est, kt))
                for pe_est, pk in pend:
                    nc.tensor.matmul(
                        yacc[:], vnat[h][:, pk, :], pe_est[:],
                        start=(pk == 0), stop=(pk == n_kt - 1),
                    )

                sums = ps.tile([1, BLK], f32, tag="xps", bufs=1, name="sums")
                nc.tensor.matmul(sums[:], ones_r[:], acc[:])
                lnr = work.tile([1, BLK], f32, tag="lnrow", bufs=2)
                nc.scalar.activation(lnr[:], sums[:], LN)
                sinvrow = work.tile([1, BLK], bf16, tag="invrow", bufs=2)
                nc.scalar.activation(sinvrow[:], lnr[:], EXP, scale=-1.0)
                sbc_ps = ps.tile([128, BLK], f32, tag="xps", bufs=1, name="sbc_ps")
                nc.tensor.matmul(sbc_ps[:], onesrow_b[:], sinvrow[:])
                sinv = work.tile([128, BLK], bf16, tag="sinv", bufs=2)
                nc.vector.tensor_copy(sinv[:], sbc_ps[:])
                ysb = work.tile([VD, BLK], bf16, tag="ysb", bufs=2)
                nc.vector.tensor_mul(out=ysb[:], in0=yacc[:], in1=sinv[:])
                for jj in range(2):
                    nc.sync.dma_start(
                        y_in[b][qc * 2 + jj, h * VD : (h + 1) * VD, :],
                        ysb[:, jj * 256 : (jj + 1) * 256],
                    )

            def emit_a2a(b):
                nc.gpsimd.collective_compute(
                    "AllToAll",
                    mybir.AluOpType.bypass,
                    replica_groups=RG,
                    ins=[y_in[b].opt()],
                    outs=[y_out[b].opt()],
                )

            def emit_wo(b, spread=False):
                """wo projection for this batch's gathered token slice.

                spread=True (final batch, all PSUM banks free): kc-outer
                over 8 concurrent bank accumulators, so wo MMs start as the
                first AllToAll chunk lands instead of after all 16.
                """
                a2a = wop.tile([128, 16, 256], bf16, tag="a2a", bufs=1, name="a2a")
                for kc in range(16):
                    nc.gpsimd.dma_start(
                        a2a[:, kc, :],
                        y_out[b][kc // 2, (kc % 2) * 128 : (kc % 2) * 128 + 128, :],
                    )
                if spread:
                    tags = ["proj", "proj", "proj", "proj", "st", "st", "xps", "yacc"]
                    bufn = [4, 4, 4, 4, 2, 2, 1, 1]
                    pouts = [
                        ps.tile([128, 512], f32, tag=tg, bufs=bu, name="pout")
                        for tg, bu in zip(tags, bufn)
                    ]
                    for kc in range(16):
                        for g, pout in enumerate(pouts):
                            tt, n = g % 2, g // 2
                            nc.tensor.matmul(
                                pout[:],
                                a2a[:, kc, tt * 128 : (tt + 1) * 128],
                                wo_sb[:, kc, n * 512 : (n + 1) * 512],
                                start=(kc == 0),
                                stop=(kc == 15),
                            )
                    for g, pout in enumerate(pouts):
                        tt, n = g % 2, g // 2
                        osb = wop.tile([128, 512], bf16, tag="osb", bufs=2)
                        nc.vector.tensor_copy(osb[:], pout[:])
                        nc.sync.dma_start(
                            out_d[
                                b, tt * 128 : (tt + 1) * 128, n * 512 : (n + 1) * 512
                            ],
                            osb[:],
                        )
                    return
                for n in range(4):
                    for tt in range(2):
                        pout = ps.tile([128, 512], f32, tag="yacc", bufs=1, name="pout")
                        for kc in range(16):
                            nc.tensor.matmul(
                                pout[:],
                                a2a[:, kc, tt * 128 : (tt + 1) * 128],
                                wo_sb[:, kc, n * 512 : (n + 1) * 512],
                                start=(kc == 0),
                                stop=(kc == 15),
                            )
                        osb = wop.tile([128, 512], bf16, tag="osb", bufs=2)
                        nc.vector.tensor_copy(osb[:], pout[:])
                        nc.sync.dma_start(
                            out_d[
                                b, tt * 128 : (tt + 1) * 128, n * 512 : (n + 1) * 512
                            ],
                            osb[:],
                        )

            # ---- software-pipelined schedule ----
            kv0 = alloc_kv(0)
            kv1 = alloc_kv(1)
            q00 = stage_a1(0, 0)
            q01 = stage_a1(0, 1)
            load_msk()
            q02 = stage_a1(0, 2)
            q03 = stage_a1(0, 3)
            load_wo()
            stage_a2(0, 0, kv0)
            stage_bh(0, 0, 0, q00, kv0)
            stage_a2(0, 1, kv0)
            stage_bh(0, 0, 1, q00, kv0)
            stage_bh(0, 1, 0, q01, kv0)
            stage_a2(0, 2, kv0)
            stage_bh(0, 1, 1, q01, kv0)
            stage_bh(0, 2, 0, q02, kv0)
            stage_a2(0, 3, kv0)
            stage_bh(0, 2, 1, q02, kv0)
            stage_bh(0, 3, 0, q03, kv0)
            stage_bh(0, 3, 1, q03, kv0)
            emit_a2a(0)
            q10 = stage_a1(1, 0)
            q11 = stage_a1(1, 1)
            q12 = stage_a1(1, 2)
            q13 = stage_a1(1, 3)
            stage_a2(1, 0, kv1)
            emit_wo(0)
            stage_bh(1, 0, 0, q10, kv1)
            stage_a2(1, 1, kv1)
            stage_bh(1, 0, 1, q10, kv1)
            stage_bh(1, 1, 0, q11, kv1)
            stage_a2(1, 2, kv1)
            stage_bh(1, 1, 1, q11, kv1)
            stage_bh(1, 2, 0, q12, kv1)
            stage_a2(1, 3, kv1)
            stage_bh(1, 2, 1, q12, kv1)
            stage_bh(1, 3, 0, q13, kv1)
            stage_bh(1, 3, 1, q13, kv1)
            emit_a2a(1)
            emit_wo(1, spread=True)

    nc.compile()
    return nc


def host_prep(x, wq, wkv_a, wkv_b, wo, kv_norm_w):
    bf = ml_dtypes.bfloat16
    scale = np.float32(QKD ** -0.5)
    inv = (1.0 / (10000.0 ** (np.arange(0, ROPE, 2, dtype=np.float32) / ROPE))).astype(
        np.float32
    )
    f = np.outer(np.arange(T, dtype=np.float32), inv)
    cos32 = np.cos(f).T.astype(np.float32)
    sin32 = np.sin(f).T.astype(np.float32)
    cos128 = np.ascontiguousarray(np.concatenate([cos32] * 4, 0)).astype(bf)
    sin128 = np.ascontiguousarray(np.concatenate([sin32] * 4, 0)).astype(bf)
    wkv_bw = (wkv_b * kv_norm_w[:, None]).astype(np.float32)
    xt = np.ascontiguousarray(x.reshape(B * T, D).T).astype(bf)
    wo_c = np.ascontiguousarray(wo).astype(bf)
    wq_r = wq.reshape(D, H, QKD)

    kk = np.arange(128)[:, None]
    qq = np.arange(BLK)[None, :]
    msk = np.concatenate(
        [(qq >= kk + j * 128).astype(np.float32) for j in range(4)], axis=1
    ).astype(bf)

    in_maps = []
    for c in range(NCORES):
        h0 = HPC * c
        w1 = np.concatenate(
            [
                wq_r[:, h0, :NOPE] * scale,
                wq_r[:, h0 + 1, :NOPE] * scale,
                wq_r[:, h0, NOPE:] * scale,
                wq_r[:, h0 + 1, NOPE:] * scale,
                wkv_a[:, c * KVS : (c + 1) * KVS],
            ],
            axis=1,
        ).astype(bf)
        wb = np.ascontiguousarray(
            wkv_bw[:, h0 * (NOPE + VD) : (h0 + 2) * (NOPE + VD)]
        ).astype(bf)
        in_maps.append(
            {
                "xt": xt,
                "w1": np.ascontiguousarray(w1),
                "wb": wb,
                "wo": wo_c,
                "cos": cos128,
                "sin": sin128,
                "msk": msk,
            }
        )
    return in_maps


_NC = None


def kernel(x, wq, wkv_a, wkv_b, wo, kv_norm_w, _trace=False):
    global _NC
    if _NC is None:
        _NC = build_program()
    in_maps = host_prep(
        np.asarray(x, np.float32),
        np.asarray(wq, np.float32),
        np.asarray(wkv_a, np.float32),
        np.asarray(wkv_b, np.float32),
        np.asarray(wo, np.float32),
        np.asarray(kv_norm_w, np.float32),
    )
    res = run_bass_kernel_spmd(_NC, in_maps, list(range(NCORES)), trace=_trace)
    out = np.empty((B, T, D), np.float32)
    cw = T // NCORES
    for c in range(NCORES):
        oc = res.results[c]["out"].astype(np.float32)  # (B, 256, D)
        for b in range(B):
            out[b, c * cw : (c + 1) * cw, :] = oc[b]
    kernel.last_results = res
    return out
